# revision 6
# baseline (speedup 1.0000x reference)
"""Two-layer DGL-style GCN on 8 Trainium2 NeuronCores.

Strategy (graph/data parallel, per sharding hint):
- Nodes are sharded 8 ways by destination; each core owns N/8 dst nodes and
  all edges pointing into them (host-side integer preprocessing).
- Aggregate-then-project: the layer is computed as
      out = relu( rin * ((A @ (rout * x)) @ W) + b )
  so the device gathers pre-scaled feature rows (x~ = rout * x, the scaling
  folded into the previous layer's flush / host input prep). This removes
  the per-layer projection pass over the full 40k-node table entirely: each
  core projects only its own 128-node dst blocks after aggregation (one
  128x128 matmul per block), which cuts both the x^T stream-in and the
  h-table write-back of the old design.
- The feature table lives in DRAM in fp16 (node-major). Dst nodes are
  sorted by in-degree into 128-node blocks; chunk t of a block holds every
  node's t-th in-edge message (pad -> zero row N). Each chunk is fetched
  with one indirect DMA (128 rows, one index per partition — the only
  offset shape the SWDGE ucode supports).
- Chunk messages accumulate transposed in PSUM (lhsT=msg, rhs=identity ->
  aggT[feat, node]), making the block aggregate directly usable as lhsT for
  the @W projection matmul. The flush applies rsqrt_in, bias, relu and (for
  layer 1) the next layer's rsqrt_out pre-scaling, emitting fp16 rows that
  are exactly the next layer's gather table.
- fp16 messages double the PE accumulate rate (1 cycle/row vs 4 for fp32)
  and halve table traffic; accumulation stays exact in fp32 PSUM.
"""
import sys

sys.path.insert(0, "/opt/trn_rl_repo")
import numpy as np
import jax
from jax.sharding import Mesh, PartitionSpec
from jax.experimental.shard_map import shard_map

import concourse.bass as bass
import concourse.mybir as mybir
import concourse.tile as tile
from concourse.bass2jax import _bass_exec_p, partition_id_tensor, install_neuronx_cc_hook

P = 128
N_CORES = 8
F = 128                                # feature dim


# ----------------------------------------------------------------------------
# harness plumbing
# ----------------------------------------------------------------------------
def _split_multiwait(nc):
    """This walrus build accepts only one sync-wait per instruction; hoist
    extras onto NoOp carriers placed immediately before."""
    for blk in nc.m.functions[0].blocks:
        new_list, changed = [], False
        for i in list(blk.instructions):
            si = i.sync_info
            if si is not None and si.on_wait and len(si.on_wait) > 1:
                waits = list(si.on_wait)
                for k, w in enumerate(waits[:-1]):
                    c = mybir.InstNoOp(name=f"{i.name}-wsplit{k}", ins=[], outs=[])
                    c.engine = i.engine
                    c.sync_info = mybir.SyncInfo(on_wait=[w], on_update=[])
                    new_list.append(c)
                si.on_wait = [waits[-1]]
                i.sync_info = si
                changed = True
            new_list.append(i)
        if changed:
            blk.instructions = new_list
    return nc


class _Runner:
    def __init__(self, nc, n_cores):
        install_neuronx_cc_hook()
        _split_multiwait(nc)
        self.n_cores = n_cores
        partition_name = nc.partition_id_tensor.name if nc.partition_id_tensor else None
        in_names, out_names, out_avals, zero_outs = [], [], [], []
        for alloc in nc.m.functions[0].allocations:
            if not isinstance(alloc, mybir.MemoryLocationSet):
                continue
            name = alloc.memorylocations[0].name
            if alloc.kind == "ExternalInput":
                if name != partition_name:
                    in_names.append(name)
            elif alloc.kind == "ExternalOutput":
                shape = tuple(alloc.tensor_shape)
                dtype = mybir.dt.np(alloc.dtype)
                out_names.append(name)
                out_avals.append(jax.core.ShapedArray(shape, dtype))
                zero_outs.append(np.zeros(shape, dtype))
        self.in_names, self.out_names = in_names, out_names
        self.out_avals, self.zero_outs = out_avals, zero_outs
        all_in_names = in_names + out_names
        if partition_name is not None:
            all_in_names.append(partition_name)

        def _body(*args):
            operands = list(args)
            if partition_name is not None:
                operands.append(partition_id_tensor())
            outs = _bass_exec_p.bind(
                *operands,
                out_avals=tuple(out_avals),
                in_names=tuple(all_in_names),
                out_names=tuple(out_names),
                lowering_input_output_aliases=(),
                sim_require_finite=False,
                sim_require_nnan=False,
                nc=nc,
            )
            return tuple(outs)

        devices = jax.devices()[:n_cores]
        mesh = Mesh(np.asarray(devices), ("core",))
        n_outs = len(out_names)
        in_specs = (PartitionSpec("core"),) * (len(in_names) + n_outs)
        out_specs = (PartitionSpec("core"),) * n_outs
        self.fn = jax.jit(
            shard_map(_body, mesh=mesh, in_specs=in_specs,
                      out_specs=out_specs, check_rep=False),
            keep_unused=True,
        )

    def run(self, in_maps):
        concat_in = [
            np.concatenate([np.asarray(in_maps[c][n]) for c in range(self.n_cores)], axis=0)
            for n in self.in_names
        ]
        concat_zeros = [
            np.zeros((self.n_cores * z.shape[0], *z.shape[1:]), z.dtype)
            for z in self.zero_outs
        ]
        outs = self.fn(*concat_in, *concat_zeros)
        jax.block_until_ready(outs)
        res = []
        for c in range(self.n_cores):
            m = {}
            for i, name in enumerate(self.out_names):
                m[name] = np.asarray(outs[i]).reshape(
                    self.n_cores, *self.out_avals[i].shape)[c]
            res.append(m)
        return res


# ----------------------------------------------------------------------------
# host-side graph preprocessing
# ----------------------------------------------------------------------------
class _Layout:
    pass


def _prep(edge_src, edge_dst, n_nodes):
    """Per-core padded-CSR layout: nodes sorted by in-degree (desc), grouped
    into 128-node blocks; chunk t of block b holds every node's t-th in-edge
    (pad -> zero row N). Chunk counts per block are shared across cores."""
    N = n_nodes
    SH = N // N_CORES
    NB = (SH + P - 1) // P
    lo = _Layout()
    deg_out = np.bincount(edge_src, minlength=N).astype(np.float32)
    deg_in_g = np.bincount(edge_dst, minlength=N).astype(np.float32)
    lo.rs_out = (1.0 / np.sqrt(np.maximum(deg_out, 1.0))).astype(np.float32)

    per_core = []
    Lb_all = np.zeros((N_CORES, NB), dtype=np.int64)
    for c in range(N_CORES):
        sel = (edge_dst >= c * SH) & (edge_dst < (c + 1) * SH)
        src_c = edge_src[sel]
        dst_c = edge_dst[sel] - c * SH
        counts = np.bincount(dst_c, minlength=SH)
        order_nodes = np.argsort(-counts, kind="stable")      # degree desc
        inv_perm = np.empty(SH, dtype=np.int64)
        inv_perm[order_nodes] = np.arange(SH)
        cs_pad = np.zeros(NB * P, dtype=np.int64)
        cs_pad[:SH] = counts[order_nodes]
        Lb_all[c] = cs_pad.reshape(NB, P).max(axis=1)
        per_core.append((src_c, dst_c, counts, order_nodes, inv_perm))

    Lb = Lb_all.max(axis=0)                                   # common chunk counts
    chunk_base = np.zeros(NB + 1, dtype=np.int64)
    np.cumsum(Lb, out=chunk_base[1:])
    lo.nchunk = int(chunk_base[-1])
    lo.lb = Lb
    lo.node_tot = NB * P
    lo.nb = NB
    lo.n = N

    gidx = np.full((N_CORES, P, lo.nchunk), N, dtype=np.int32)
    rin_t = np.ones((N_CORES, P, NB), dtype=np.float32)
    srow_t = np.ones((N_CORES, P, NB), dtype=np.float32)
    node_of_pos = np.full((N_CORES, NB * P), -1, dtype=np.int64)
    for c in range(N_CORES):
        src_c, dst_c, counts, order_nodes, inv_perm = per_core[c]
        node_of_pos[c, :SH] = order_nodes + c * SH
        order = np.argsort(dst_c, kind="stable")
        ds = dst_c[order]
        ss = src_c[order]
        starts = np.zeros(SH + 1, dtype=np.int64)
        np.cumsum(counts, out=starts[1:])
        t_idx = np.arange(len(ds)) - starts[ds]               # edge rank within node
        pos = inv_perm[ds]
        q = chunk_base[pos // P] + t_idx
        gidx[c, pos % P, q] = ss
        nid = node_of_pos[c]
        valid = nid >= 0
        ri = np.ones(NB * P, dtype=np.float32)
        ri[valid] = 1.0 / np.sqrt(np.maximum(deg_in_g[nid[valid]], 1.0))
        rin_t[c] = ri.reshape(NB, P).T
        so = np.ones(NB * P, dtype=np.float32)
        so[valid] = lo.rs_out[nid[valid]]
        srow_t[c] = so.reshape(NB, P).T

    lo.gidx = gidx
    lo.rin = rin_t
    lo.srow = srow_t
    lo.node_of_pos = node_of_pos
    return lo


# ----------------------------------------------------------------------------
# device kernel
# ----------------------------------------------------------------------------
def _build_nc(lo, repeat=1):
    N, NB, NCHUNK = lo.n, lo.nb, lo.nchunk
    NODE_TOT = lo.node_tot
    nc = bass.Bass(num_swdge_queues=4)
    tc = tile.TileContext(nc)
    f32 = mybir.dt.float32
    f16 = mybir.dt.float16

    xtab = nc.dram_tensor("xtab", [N + 1, F], f16, kind="ExternalInput")
    W = nc.dram_tensor("W", [P, F], f16, kind="ExternalInput")
    identd = nc.dram_tensor("identd", [P, P], f16, kind="ExternalInput")
    brow = nc.dram_tensor("brow", [1, F], f32, kind="ExternalInput")
    rin = nc.dram_tensor("rin", [P, NB], f32, kind="ExternalInput")
    srow = nc.dram_tensor("srow", [P, NB], f32, kind="ExternalInput")
    gidx = nc.dram_tensor("gidx", [P, NCHUNK], mybir.dt.int32, kind="ExternalInput")
    out = nc.dram_tensor("out", [NODE_TOT, F], f16, kind="ExternalOutput")

    with tc:
        with (
            tc.tile_pool(name="const", bufs=1) as constp,
            tc.tile_pool(name="msg", bufs=16) as msgp,
            tc.tile_pool(name="hsb", bufs=4) as hsbp,
            tc.tile_pool(name="osb", bufs=6) as osbp,
            tc.tile_pool(name="apsum", bufs=5, space="PSUM") as apsum,
            tc.tile_pool(name="ypsum", bufs=2, space="PSUM") as ypsum,
            tc.tile_pool(name="bpsum", bufs=1, space="PSUM") as bpsum,
        ):
            # ---- constants
            W_sb = constp.tile([P, F], f16)
            nc.sync.dma_start(W_sb[:], W[:])
            ident = constp.tile([P, P], f16)
            nc.sync.dma_start(ident[:], identd[:])
            gidx_sb = constp.tile([P, NCHUNK], mybir.dt.int32)
            nc.sync.dma_start(gidx_sb[:], gidx[:])
            brow_sb = constp.tile([1, F], f32)
            nc.sync.dma_start(brow_sb[:], brow[:])
            rin_sb = constp.tile([P, NB], f32)
            nc.sync.dma_start(rin_sb[:], rin[:])
            srow_sb = constp.tile([P, NB], f32)
            nc.sync.dma_start(srow_sb[:], srow[:])

            ones1 = constp.tile([1, F], f32)
            nc.vector.memset(ones1[:], 1.0)
            bps = bpsum.tile([P, F], f32)
            nc.tensor.matmul(out=bps[:], lhsT=ones1[:], rhs=brow_sb[:],
                             start=True, stop=True)
            b_bcast = constp.tile([P, F], f32)
            nc.vector.tensor_copy(b_bcast[:], bps[:])

            for _rep in range(repeat):
                def flush_block(b, y):
                    o1 = osbp.tile([P, F], f32)
                    nc.vector.tensor_scalar(o1[:], y[:], rin_sb[:, b:b + 1], None,
                                            mybir.AluOpType.mult)
                    nc.vector.tensor_tensor(o1[:], o1[:], b_bcast[:],
                                            op=mybir.AluOpType.add)
                    o2 = osbp.tile([P, F], f16)
                    nc.scalar.activation(o2[:], o1[:],
                                         mybir.ActivationFunctionType.Relu,
                                         scale=srow_sb[:, b:b + 1])
                    nc.sync.dma_start(out[b * P:(b + 1) * P, :], o2[:])

                q = 0
                for b in range(NB):
                    T = int(lo.lb[b])
                    if T == 0:
                        o1 = osbp.tile([P, F], f32)
                        nc.vector.tensor_copy(o1[:], b_bcast[:])
                        o2 = osbp.tile([P, F], f16)
                        nc.scalar.activation(o2[:], o1[:],
                                             mybir.ActivationFunctionType.Relu,
                                             scale=srow_sb[:, b:b + 1])
                        nc.sync.dma_start(out[b * P:(b + 1) * P, :], o2[:])
                        continue
                    aggT = apsum.tile([P, F], f32)
                    for t in range(T):
                        mt = msgp.tile([P, F], f16)
                        inst = nc.gpsimd.indirect_dma_start(
                            out=mt[:, :],
                            out_offset=None,
                            in_=xtab[:],
                            in_offset=bass.IndirectOffsetOnAxis(
                                ap=gidx_sb[:, q:q + 1], axis=0),
                        )
                        qi = q % 4
                        inst.ins.queue = f"qPoolDynamic{qi if qi else ''}"
                        nc.tensor.matmul(out=aggT[:], lhsT=mt[:, :], rhs=ident[:],
                                         start=(t == 0), stop=(t == T - 1))
                        q += 1
                    aggT_sb = hsbp.tile([P, F], f16)
                    nc.scalar.activation(aggT_sb[:], aggT[:],
                                         mybir.ActivationFunctionType.Copy)
                    y = ypsum.tile([P, F], f32)
                    nc.tensor.matmul(out=y[:], lhsT=aggT_sb[:], rhs=W_sb[:],
                                     start=True, stop=True)
                    flush_block(b, y)
    return nc


# ----------------------------------------------------------------------------
# public entry
# ----------------------------------------------------------------------------
_CACHE = {}


def _get_runner(edge_src, edge_dst, n_nodes):
    key = (n_nodes, edge_src.shape[0],
           int(edge_src[::997].astype(np.int64).sum()),
           int(edge_dst[::997].astype(np.int64).sum()))
    if key not in _CACHE:
        lo = _prep(edge_src, edge_dst, n_nodes)
        nc = _build_nc(lo)
        _CACHE[key] = (lo, _Runner(nc, N_CORES))
    return _CACHE[key]


_IDENT16 = np.eye(P, dtype=np.float16)


def _in_maps(lo, xbase16, W, b, layer):
    """Per-core input dicts from a node-order fp16 base table [N, F].
    layer=1 applies next-layer rsqrt_out pre-scaling in the flush."""
    W16 = np.ascontiguousarray(W.astype(np.float16))
    brow = np.ascontiguousarray(b.astype(np.float32)[None, :])
    tab = np.zeros((lo.n + 1, F), dtype=np.float16)
    tab[:lo.n] = xbase16
    maps = []
    for c in range(N_CORES):
        maps.append({
            "xtab": tab,
            "W": W16,
            "identd": _IDENT16,
            "brow": brow,
            "rin": lo.rin[c],
            "srow": lo.srow[c] if layer == 1 else np.ones_like(lo.srow[c]),
            "gidx": lo.gidx[c],
        })
    return maps


def _scatter_out(lo, res, dtype):
    full = np.zeros((lo.n, F), dtype=dtype)
    for c in range(N_CORES):
        nid = lo.node_of_pos[c]
        valid = nid >= 0
        full[nid[valid]] = res[c]["out"][valid]
    return full


def kernel(features, edge_src, edge_dst, W1, b1, W2, b2):
    features = np.asarray(features, dtype=np.float32)
    edge_src = np.asarray(edge_src, dtype=np.int32)
    edge_dst = np.asarray(edge_dst, dtype=np.int32)
    n = features.shape[0]
    lo, runner = _get_runner(edge_src, edge_dst, n)

    x1 = (features * lo.rs_out[:, None]).astype(np.float16)
    res1 = runner.run(_in_maps(lo, x1, np.asarray(W1), np.asarray(b1), layer=1))

    x2 = _scatter_out(lo, res1, np.float16)       # already rsqrt_out-scaled
    res2 = runner.run(_in_maps(lo, x2, np.asarray(W2), np.asarray(b2), layer=2))
    return _scatter_out(lo, res2, np.float32)
